# revision 12
# baseline (speedup 1.0000x reference)
"""Trainium2 Bass kernel for ConstrainedAttentionModel (sparse_attention).

Full-input contract: kernel(x=[8,2048] int, C=[4,4] f32) -> [8,2048] f32.
Data parallel across 8 NeuronCores: one batch row per core.

Math (per row, T=2048, k=4, V=2048):
  scores[t] = sum_{i,j} C[i,j] * [x[t-j] == x[T-1-i]]   (t-j >= 0)
  scores[T-1] = -inf; attn = softmax(scores)
  out[v] = sum_t attn[t] * [x[t] == v]

v3 design (t = 16p + f layout on 128 partitions):
  - ONE input DMA: the host packs a per-partition image holding the
    fp16 x-window (20 wide), queries replicated across the window,
    C (re-ordered for the conv view), the softmax-mask bias row,
    the base-64 digits of x (lo=x&63, hi=x>>6), the class iotas and
    ones rows
  - windowed equality m[p,i,e] = [x_win[p,e]==q_i], conv view with C
    -> scores; mask folded in as a 17th reduce channel copied from
    the image by the scalar engine
  - exp on the scalar engine in two f-halves with fused row-sum
    accumulation, so the E-weighting of the first half starts early
  - vocab one-hot factorized v = 64*hi + lo in fp16; out[hi,lo] =
    sum_f A_f^T @ B_f as 16 fp16 PSUM-accumulated matmuls
  - sync=False scheduler edges force the DVE score chain ahead of
    the one-hot builds (the greedy list scheduler would otherwise
    interleave them and delay exp by ~1.5us)
  - sum(E) replicated onto the 32 output partitions by a ones-matmul,
    reciprocal on DVE, applied directly to the PSUM accumulator
"""
import numpy as np
import concourse.bass as bass
import concourse.bacc as bacc
import concourse.tile as tile
from concourse import mybir
from concourse.tile_rust import add_dep_helper

P = 128
T = 2048
F = T // P  # 16
K = 4
FH = F // 2  # 8
NHI = 32
NLO = 64
WIN = 20  # x-window width per partition (19 used, padded to 20)
NEG = -60000.0  # large-negative mask bias, exactly representable in fp16

fp32 = mybir.dt.float32
fp16 = mybir.dt.float16
i32 = mybir.dt.int32
Alu = mybir.AluOpType
Act = mybir.ActivationFunctionType

# int32-word offsets inside the packed per-partition image
OFF_XW = 0  # [20] fp16 x-window          -> 10 words
OFF_QR = 10  # [4,20] fp16 query replicas   -> 40 words
OFF_CR = 50  # [16] fp16 C (i,jj) order     ->  8 words
OFF_BIAS = 58  # [16] fp16 mask bias row      ->  8 words
OFF_XLO = 66  # [16] fp16 x & 63             ->  8 words
OFF_XHI = 74  # [16] fp16 x >> 6             ->  8 words
OFF_IL = 82  # [64] fp16 iota 0..63         -> 32 words
OFF_IH = 114  # [32] fp16 iota 0..31         -> 16 words
OFF_ONE = 130  # [1] fp32 ones column         ->  1 word
OFF_ONR = 131  # [32] fp32 ones row           -> 32 words
IMG_W = 163

B = 8


def _build_nc():
    nc = bacc.Bacc()
    img = nc.dram_tensor("img", [P, IMG_W], i32, kind="ExternalInput")
    y = nc.dram_tensor("y", [T], fp32, kind="ExternalOutput")

    with tile.TileContext(nc) as tc:
        with (
            tc.tile_pool(name="sb", bufs=1) as sb,
            tc.tile_pool(name="ps", bufs=1, space="PSUM") as ps,
        ):
            IMGT = sb.tile([P, IMG_W], i32)
            # split by partition halves: disjoint SDMA engine sets, and the
            # two HWDGE rings (sync/scalar) generate descriptors in parallel
            nc.sync.dma_start(out=IMGT[0 : P // 2, :], in_=img[0 : P // 2, :])
            nc.scalar.dma_start(out=IMGT[P // 2 :, :], in_=img[P // 2 :, :])

            xw = IMGT[:, OFF_XW : OFF_XW + 10].bitcast(fp16)  # [P, 20]
            qr = IMGT[:, OFF_QR : OFF_QR + 40].bitcast(fp16).rearrange(
                "p (i e) -> p i e", e=WIN
            )  # [P, 4, 20]
            cr = IMGT[:, OFF_CR : OFF_CR + 8].bitcast(fp16).rearrange(
                "p (i jj) -> p i jj", jj=K
            )  # [P, 4, 4]
            bias = IMGT[:, OFF_BIAS : OFF_BIAS + 8].bitcast(fp16)  # [P, 16]
            xlo = IMGT[:, OFF_XLO : OFF_XLO + 8].bitcast(fp16)  # [P, 16]
            xhi = IMGT[:, OFF_XHI : OFF_XHI + 8].bitcast(fp16)  # [P, 16]
            il = IMGT[:, OFF_IL : OFF_IL + 32].bitcast(fp16)  # [P, 64]
            ih = IMGT[:, OFF_IH : OFF_IH + 16].bitcast(fp16)  # [P, 32]
            onec = IMGT[:, OFF_ONE : OFF_ONE + 1].bitcast(fp32)  # [P, 1]
            oner = IMGT[:, OFF_ONR : OFF_ONR + 32].bitcast(fp32)  # [P, 32]

            EQ = sb.tile([P, K, WIN], fp16)  # m[p,i,e] = [xw[p,e]==q_i]
            CE = sb.tile([P, F, 17], fp16)  # c<16: C*m products, c=16: bias
            SC = sb.tile([P, F], fp32)
            E = sb.tile([P, F], fp16)
            RS = sb.tile([P, 2], fp32)
            AEQ = sb.tile([P, F, NHI], fp16)
            BT = sb.tile([P, F, NLO], fp16)
            A = sb.tile([P, F, NHI], fp16)
            RINV = sb.tile([NHI, 1], fp32)
            OUT = sb.tile([NHI, NLO], fp32)
            acc = ps.tile([NHI, NLO], fp32)
            S1 = ps.tile([NHI, 1], fp32)

            h0 = slice(0, FH)
            h1 = slice(FH, F)

            # mask bias -> 17th reduce channel (scalar engine, off critical path)
            nc.scalar.activation(out=CE[:, :, 16], in_=bias, func=Act.Copy)

            # ---- score chain (must run first on DVE) ----
            nc.vector.tensor_tensor(
                out=EQ[:],
                in0=xw[:, None, :].broadcast_to([P, K, WIN]),
                in1=qr,
                op=Alu.is_equal,
            )
            eq = EQ[:]
            EQV = bass.AP(
                tensor=eq.tensor,
                offset=eq.offset,
                ap=[eq.ap[0], [1, F], [WIN, K], [1, K]],
            )  # [P, f, i, jj] = m[p, i, f+jj]
            nc.vector.tensor_tensor(
                out=CE[:, :, 0:16].rearrange("p f (i jj) -> p f i jj", jj=K),
                in0=EQV,
                in1=cr[:, None, :, :].broadcast_to([P, F, K, K]),
                op=Alu.mult,
            )
            red = nc.vector.reduce_sum(
                out=SC[:], in_=CE[:], axis=mybir.AxisListType.X
            )
            # E = exp(scores) in halves; RS = per-partition sums
            nc.scalar.activation(
                out=E[:, h0], in_=SC[:, h0], func=Act.Exp,
                accum_out=RS[:, 0:1],
            )
            nc.scalar.activation(
                out=E[:, h1], in_=SC[:, h1], func=Act.Exp,
                accum_out=RS[:, 1:2],
            )
            # S = sum_p RS, replicated onto all 32 output partitions by the
            # ones lhsT; the two halves accumulate in PSUM
            nc.tensor.matmul(
                S1[:], lhsT=oner, rhs=RS[:, 0:1], start=True, stop=False,
                skip_group_check=True,
            )
            nc.tensor.matmul(
                S1[:], lhsT=oner, rhs=RS[:, 1:2], start=False, stop=True,
                skip_group_check=True,
            )

            # ---- one-hot builds + weighting + accumulating outer products ----
            def after_scores(bi):
                add_dep_helper(
                    bi.ins, red.ins, sync=False, reason="score chain first"
                )

            a0 = nc.vector.tensor_tensor(
                out=AEQ[:, h0],
                in0=xhi[:, h0][:, :, None].broadcast_to([P, FH, NHI]),
                in1=ih[:, None, :].broadcast_to([P, FH, NHI]),
                op=Alu.is_equal,
            )
            after_scores(a0)
            b0 = nc.vector.tensor_tensor(
                out=BT[:, h0],
                in0=xlo[:, h0][:, :, None].broadcast_to([P, FH, NLO]),
                in1=il[:, None, :].broadcast_to([P, FH, NLO]),
                op=Alu.is_equal,
            )
            after_scores(b0)
            nc.vector.tensor_tensor(
                out=A[:, h0],
                in0=AEQ[:, h0],
                in1=E[:, h0][:, :, None].broadcast_to([P, FH, NHI]),
                op=Alu.mult,
            )
            for f in range(0, FH):
                nc.tensor.matmul(
                    acc[:],
                    lhsT=A[:, f, :],
                    rhs=BT[:, f, :],
                    start=(f == 0),
                    stop=False,
                    skip_group_check=True,
                )
            a1 = nc.vector.tensor_tensor(
                out=AEQ[:, h1],
                in0=xhi[:, h1][:, :, None].broadcast_to([P, FH, NHI]),
                in1=ih[:, None, :].broadcast_to([P, FH, NHI]),
                op=Alu.is_equal,
            )
            after_scores(a1)
            b1 = nc.vector.tensor_tensor(
                out=BT[:, h1],
                in0=xlo[:, h1][:, :, None].broadcast_to([P, FH, NLO]),
                in1=il[:, None, :].broadcast_to([P, FH, NLO]),
                op=Alu.is_equal,
            )
            after_scores(b1)
            nc.vector.tensor_tensor(
                out=A[:, h1],
                in0=AEQ[:, h1],
                in1=E[:, h1][:, :, None].broadcast_to([P, FH, NHI]),
                op=Alu.mult,
            )
            for f in range(FH, F):
                nc.tensor.matmul(
                    acc[:],
                    lhsT=A[:, f, :],
                    rhs=BT[:, f, :],
                    start=False,
                    stop=(f == F - 1),
                    skip_group_check=True,
                )

            # ---- 1/S, scale, store ----
            nc.vector.reciprocal(out=RINV[:], in_=S1[:])
            nc.vector.tensor_scalar(
                out=OUT[:], in0=acc[:], scalar1=RINV[:], scalar2=None, op0=Alu.mult
            )
            yv = y[:].rearrange("(h l) -> h l", l=NLO)
            nc.sync.dma_start(out=yv[0 : NHI // 2, :], in_=OUT[0 : NHI // 2, :])
            nc.scalar.dma_start(out=yv[NHI // 2 :, :], in_=OUT[NHI // 2 :, :])
    nc.compile()
    return nc


def _host_prep(x_row: np.ndarray, C: np.ndarray):
    x_row = x_row.astype(np.int32)
    xpad = np.concatenate(
        [np.full(K - 1, -1, np.int32), x_row, np.full(1, -1, np.int32)]
    )
    idx = 16 * np.arange(P)[:, None] + np.arange(WIN)[None, :]
    xw = xpad[idx].astype(np.float16)  # [128, 20]
    q = x_row[T - 1 : T - 1 - K : -1].astype(np.float16)  # q[i] = x[T-1-i]
    qrep = np.tile(q[:, None], (1, WIN)).reshape(-1)  # [80]
    cr = np.ascontiguousarray(C[:, ::-1]).astype(np.float16).reshape(-1)  # [16]
    bias = np.zeros((P, F), np.float16)
    bias[P - 1, F - 1] = NEG
    xt = x_row.reshape(P, F)
    xlo = (xt & 63).astype(np.float16)
    xhi = (xt >> 6).astype(np.float16)
    il = np.arange(NLO, dtype=np.float16)
    ih = np.arange(NHI, dtype=np.float16)
    onec = np.ones(1, np.float32)
    oner = np.ones(NHI, np.float32)

    img = np.empty((P, IMG_W * 4), np.uint8)
    for p in range(P):
        row = np.concatenate(
            [
                xw[p].view(np.uint8),
                qrep.view(np.uint8),
                cr.view(np.uint8),
                bias[p].view(np.uint8),
                xlo[p].view(np.uint8),
                xhi[p].view(np.uint8),
                il.view(np.uint8),
                ih.view(np.uint8),
                onec.view(np.uint8),
                oner.view(np.uint8),
            ]
        )
        img[p] = row
    return {"img": img.view(np.int32)}


_NC_CACHE = {}


def _get_nc():
    if "nc" not in _NC_CACHE:
        _NC_CACHE["nc"] = _build_nc()
    return _NC_CACHE["nc"]


def kernel(x: np.ndarray, C: np.ndarray, _spmd_kwargs: dict | None = None):
    from concourse.bass_utils import run_bass_kernel_spmd

    x = np.asarray(x).astype(np.int32)  # token ids < 2048, exact
    C = np.asarray(C).astype(np.float32)
    assert x.shape == (B, T) and C.shape == (K, K)
    in_maps = [_host_prep(x[b], C) for b in range(B)]
    res = run_bass_kernel_spmd(
        _get_nc(), in_maps, core_ids=list(range(B)), **(_spmd_kwargs or {})
    )
    out = np.stack([res.results[b]["y"] for b in range(B)], axis=0)
    if _spmd_kwargs:
        kernel.last_results = res
    return out


# revision 16
# speedup vs baseline: 1.1012x; 1.1012x over previous
"""Trainium2 Bass kernel for ConstrainedAttentionModel (sparse_attention).

Full-input contract: kernel(x=[8,2048] int, C=[4,4] f32) -> [8,2048] f32.
Data parallel across 8 NeuronCores: one batch row per core.

Math (per row, T=2048, k=4, V=2048):
  scores[t] = sum_{i,j} C[i,j] * [x[t-j] == x[T-1-i]]   (t-j >= 0)
  scores[T-1] = -inf; attn = softmax(scores)
  out[v] = sum_t attn[t] * [x[t] == v]

v3 design (t = 16p + f layout on 128 partitions):
  - ONE input DMA: the host packs a per-partition image holding the
    fp16 x-window (20 wide), queries replicated across the window,
    C (re-ordered for the conv view), the softmax-mask bias row,
    the base-64 digits of x (lo=x&63, hi=x>>6), the class iotas and
    ones rows
  - windowed equality m[p,i,e] = [x_win[p,e]==q_i], conv view with C
    -> scores; mask folded in as a 17th reduce channel copied from
    the image by the scalar engine
  - exp on the scalar engine in two f-halves with fused row-sum
    accumulation, so the E-weighting of the first half starts early
  - vocab one-hot factorized v = 64*hi + lo in fp16; out[hi,lo] =
    sum_f A_f^T @ B_f as 16 fp16 PSUM-accumulated matmuls
  - sync=False scheduler edges force the DVE score chain ahead of
    the one-hot builds (the greedy list scheduler would otherwise
    interleave them and delay exp by ~1.5us)
  - sum(E) replicated onto the 32 output partitions by a ones-matmul,
    reciprocal on DVE, applied directly to the PSUM accumulator
"""
import os
import numpy as np
import concourse.bass as bass
import concourse.bacc as bacc
import concourse.tile as tile
from concourse import mybir
from concourse.tile_rust import add_dep_helper

P = 128
T = 2048
F = T // P  # 16
K = 4
FH = F // 2  # 8
NHI = 32
NLO = 64
WIN = 20  # x-window width per partition (19 used, padded to 20)
NEG = -60000.0  # large-negative mask bias, exactly representable in fp16

fp32 = mybir.dt.float32
fp16 = mybir.dt.float16
i32 = mybir.dt.int32
Alu = mybir.AluOpType
Act = mybir.ActivationFunctionType

# int32-word offsets inside the packed per-partition image
OFF_XW = 0  # [20] fp16 x-window          -> 10 words
OFF_QR = 10  # [4,20] fp16 query replicas   -> 40 words
OFF_CR = 50  # [16] fp16 C (i,jj) order     ->  8 words
OFF_BIAS = 58  # [16] fp16 mask bias row      ->  8 words
OFF_XLO = 66  # [16] fp16 x & 63             ->  8 words
OFF_XHI = 74  # [16] fp16 x >> 6             ->  8 words
OFF_IL = 82  # [64] fp16 iota 0..63         -> 32 words
OFF_IH = 114  # [32] fp16 iota 0..31         -> 16 words
OFF_ONE = 130  # [1] fp32 ones column         ->  1 word
OFF_ONR = 131  # [32] fp32 ones row           -> 32 words
IMG_W = 163

SPLIT_DMA = os.environ.get("KERNEL_SPLIT_DMA", "1") == "1"

B = 8


def _build_nc():
    nc = bacc.Bacc()
    img = nc.dram_tensor("img", [P, IMG_W], i32, kind="ExternalInput")
    y = nc.dram_tensor("y", [T], fp32, kind="ExternalOutput")

    with tile.TileContext(nc) as tc:
        with (
            tc.tile_pool(name="sb", bufs=1) as sb,
            tc.tile_pool(name="ps", bufs=1, space="PSUM") as ps,
        ):
            IMGT = sb.tile([P, IMG_W], i32)
            if SPLIT_DMA:
                # split by partition halves: disjoint SDMA engine sets, and
                # the two HWDGE rings generate descriptors in parallel
                nc.sync.dma_start(
                    out=IMGT[0 : P // 2, :], in_=img[0 : P // 2, :]
                )
                nc.scalar.dma_start(
                    out=IMGT[P // 2 :, :], in_=img[P // 2 :, :]
                )
            else:
                nc.sync.dma_start(out=IMGT[:], in_=img[:])

            xw = IMGT[:, OFF_XW : OFF_XW + 10].bitcast(fp16)  # [P, 20]
            qr = IMGT[:, OFF_QR : OFF_QR + 40].bitcast(fp16).rearrange(
                "p (i e) -> p i e", e=WIN
            )  # [P, 4, 20]
            cr = IMGT[:, OFF_CR : OFF_CR + 8].bitcast(fp16).rearrange(
                "p (i jj) -> p i jj", jj=K
            )  # [P, 4, 4]
            bias = IMGT[:, OFF_BIAS : OFF_BIAS + 8].bitcast(fp16)  # [P, 16]
            xlo = IMGT[:, OFF_XLO : OFF_XLO + 8].bitcast(fp16)  # [P, 16]
            xhi = IMGT[:, OFF_XHI : OFF_XHI + 8].bitcast(fp16)  # [P, 16]
            il = IMGT[:, OFF_IL : OFF_IL + 32].bitcast(fp16)  # [P, 64]
            ih = IMGT[:, OFF_IH : OFF_IH + 16].bitcast(fp16)  # [P, 32]
            onec = IMGT[:, OFF_ONE : OFF_ONE + 1].bitcast(fp32)  # [P, 1]
            oner = IMGT[:, OFF_ONR : OFF_ONR + 32].bitcast(fp32)  # [P, 32]

            EQ = sb.tile([P, K, WIN], fp16)  # m[p,i,e] = [xw[p,e]==q_i]
            CE = sb.tile([P, F, 17], fp16)  # c<16: C*m products, c=16: bias
            SC = sb.tile([P, F], fp32)
            E = sb.tile([P, F], fp16)
            RS = sb.tile([P, 2], fp32)
            AEQ = sb.tile([P, F, NHI], fp16)
            BT = sb.tile([P, F, NLO], fp16)
            A = sb.tile([P, F, NHI], fp16)
            RINV = sb.tile([NHI, 1], fp32)
            OUT = sb.tile([NHI, NLO], fp32)
            acc = ps.tile([NHI, NLO], fp32)
            S1 = ps.tile([NHI, 1], fp32)

            h0 = slice(0, FH)
            h1 = slice(FH, F)

            # mask bias -> 17th reduce channel (scalar engine, off critical path)
            nc.scalar.activation(out=CE[:, :, 16], in_=bias, func=Act.Copy)

            # ---- score chain (must run first on DVE) ----
            nc.vector.tensor_tensor(
                out=EQ[:],
                in0=xw[:, None, :].broadcast_to([P, K, WIN]),
                in1=qr,
                op=Alu.is_equal,
            )
            eq = EQ[:]
            EQV = bass.AP(
                tensor=eq.tensor,
                offset=eq.offset,
                ap=[eq.ap[0], [1, F], [WIN, K], [1, K]],
            )  # [P, f, i, jj] = m[p, i, f+jj]
            nc.vector.tensor_tensor(
                out=CE[:, :, 0:16].rearrange("p f (i jj) -> p f i jj", jj=K),
                in0=EQV,
                in1=cr[:, None, :, :].broadcast_to([P, F, K, K]),
                op=Alu.mult,
            )
            red = nc.vector.reduce_sum(
                out=SC[:], in_=CE[:], axis=mybir.AxisListType.X
            )
            # E = exp(scores) in halves; RS = per-partition sums
            nc.scalar.activation(
                out=E[:, h0], in_=SC[:, h0], func=Act.Exp,
                accum_out=RS[:, 0:1],
            )
            nc.scalar.activation(
                out=E[:, h1], in_=SC[:, h1], func=Act.Exp,
                accum_out=RS[:, 1:2],
            )
            # S = sum_p RS, replicated onto all 32 output partitions by the
            # ones lhsT; the two halves accumulate in PSUM
            nc.tensor.matmul(
                S1[:], lhsT=oner, rhs=RS[:, 0:1], start=True, stop=False,
                skip_group_check=True,
            )
            nc.tensor.matmul(
                S1[:], lhsT=oner, rhs=RS[:, 1:2], start=False, stop=True,
                skip_group_check=True,
            )

            # ---- one-hot builds + weighting + accumulating outer products ----
            def after_scores(bi):
                add_dep_helper(
                    bi.ins, red.ins, sync=False, reason="score chain first"
                )

            a0 = nc.vector.tensor_tensor(
                out=AEQ[:, h0],
                in0=xhi[:, h0][:, :, None].broadcast_to([P, FH, NHI]),
                in1=ih[:, None, :].broadcast_to([P, FH, NHI]),
                op=Alu.is_equal,
            )
            after_scores(a0)
            b0 = nc.vector.tensor_tensor(
                out=BT[:, h0],
                in0=xlo[:, h0][:, :, None].broadcast_to([P, FH, NLO]),
                in1=il[:, None, :].broadcast_to([P, FH, NLO]),
                op=Alu.is_equal,
            )
            after_scores(b0)
            nc.vector.tensor_tensor(
                out=A[:, h0],
                in0=AEQ[:, h0],
                in1=E[:, h0][:, :, None].broadcast_to([P, FH, NHI]),
                op=Alu.mult,
            )
            for f in range(0, FH):
                nc.tensor.matmul(
                    acc[:],
                    lhsT=A[:, f, :],
                    rhs=BT[:, f, :],
                    start=(f == 0),
                    stop=False,
                    skip_group_check=True,
                )
            a1 = nc.vector.tensor_tensor(
                out=AEQ[:, h1],
                in0=xhi[:, h1][:, :, None].broadcast_to([P, FH, NHI]),
                in1=ih[:, None, :].broadcast_to([P, FH, NHI]),
                op=Alu.is_equal,
            )
            after_scores(a1)
            b1 = nc.vector.tensor_tensor(
                out=BT[:, h1],
                in0=xlo[:, h1][:, :, None].broadcast_to([P, FH, NLO]),
                in1=il[:, None, :].broadcast_to([P, FH, NLO]),
                op=Alu.is_equal,
            )
            after_scores(b1)
            nc.vector.tensor_tensor(
                out=A[:, h1],
                in0=AEQ[:, h1],
                in1=E[:, h1][:, :, None].broadcast_to([P, FH, NHI]),
                op=Alu.mult,
            )
            for f in range(FH, F):
                nc.tensor.matmul(
                    acc[:],
                    lhsT=A[:, f, :],
                    rhs=BT[:, f, :],
                    start=False,
                    stop=(f == F - 1),
                    skip_group_check=True,
                )

            # ---- 1/S, scale, store ----
            nc.vector.reciprocal(out=RINV[:], in_=S1[:])
            nc.vector.tensor_scalar(
                out=OUT[:], in0=acc[:], scalar1=RINV[:], scalar2=None, op0=Alu.mult
            )
            yv = y[:].rearrange("(h l) -> h l", l=NLO)
            if SPLIT_DMA:
                nc.sync.dma_start(
                    out=yv[0 : NHI // 2, :], in_=OUT[0 : NHI // 2, :]
                )
                nc.scalar.dma_start(
                    out=yv[NHI // 2 :, :], in_=OUT[NHI // 2 :, :]
                )
            else:
                nc.sync.dma_start(out=yv, in_=OUT[:])
    nc.compile()
    return nc


def _host_prep(x_row: np.ndarray, C: np.ndarray):
    x_row = x_row.astype(np.int32)
    xpad = np.concatenate(
        [np.full(K - 1, -1, np.int32), x_row, np.full(1, -1, np.int32)]
    )
    idx = 16 * np.arange(P)[:, None] + np.arange(WIN)[None, :]
    xw = xpad[idx].astype(np.float16)  # [128, 20]
    q = x_row[T - 1 : T - 1 - K : -1].astype(np.float16)  # q[i] = x[T-1-i]
    qrep = np.tile(q[:, None], (1, WIN)).reshape(-1)  # [80]
    cr = np.ascontiguousarray(C[:, ::-1]).astype(np.float16).reshape(-1)  # [16]
    bias = np.zeros((P, F), np.float16)
    bias[P - 1, F - 1] = NEG
    xt = x_row.reshape(P, F)
    xlo = (xt & 63).astype(np.float16)
    xhi = (xt >> 6).astype(np.float16)
    il = np.arange(NLO, dtype=np.float16)
    ih = np.arange(NHI, dtype=np.float16)
    onec = np.ones(1, np.float32)
    oner = np.ones(NHI, np.float32)

    img = np.empty((P, IMG_W * 4), np.uint8)
    for p in range(P):
        row = np.concatenate(
            [
                xw[p].view(np.uint8),
                qrep.view(np.uint8),
                cr.view(np.uint8),
                bias[p].view(np.uint8),
                xlo[p].view(np.uint8),
                xhi[p].view(np.uint8),
                il.view(np.uint8),
                ih.view(np.uint8),
                onec.view(np.uint8),
                oner.view(np.uint8),
            ]
        )
        img[p] = row
    return {"img": img.view(np.int32)}


_NC_CACHE = {}


def _get_nc():
    if "nc" not in _NC_CACHE:
        _NC_CACHE["nc"] = _build_nc()
    return _NC_CACHE["nc"]


def kernel(x: np.ndarray, C: np.ndarray, _spmd_kwargs: dict | None = None):
    from concourse.bass_utils import run_bass_kernel_spmd

    x = np.asarray(x).astype(np.int32)  # token ids < 2048, exact
    C = np.asarray(C).astype(np.float32)
    assert x.shape == (B, T) and C.shape == (K, K)
    in_maps = [_host_prep(x[b], C) for b in range(B)]
    res = run_bass_kernel_spmd(
        _get_nc(), in_maps, core_ids=list(range(B)), **(_spmd_kwargs or {})
    )
    out = np.stack([res.results[b]["y"] for b in range(B)], axis=0)
    if _spmd_kwargs:
        kernel.last_results = res
    return out
